# revision 25
# baseline (speedup 1.0000x reference)
"""Trainium2 Bass kernel for nn_Attention2 (multi-branch conv attention).

Reference computation (per batch):
  q = (hidden @ Wq.T + bq)                      -> [S, D] viewed [H, S, HD]
  for each depthwise conv branch b in {1x1, 3x3, 5x5} (VALID padding):
    xb = concat([cls, conv_b(patches)])         -> [L_b, D]
    k = xb @ Wk.T + bk ; v = xb @ Wv.T + bv
    ctx += softmax(q k^T / sqrt(HD)) @ v
  out = (ctx / 3) @ Wo.T + bo

Sharding: data-parallel over batch, 8 batches per core on 8 cores.

Layout strategy inside a core (all per-batch):
  - hidden transposed on PE -> hT [c=128 x 6, tokens] (f32r)
  - Q/K projections in T-layout (features on partitions), fp32r matmuls
  - scores computed transposed: sT[k, q] = K_head @ Q_head^T so that the
    probs tile is directly usable as the stationary operand of the ctx
    matmul (no probs transpose needed); softmax denominator comes free via
    an extra ones-column appended to V (per-head 65-column layout).
  - V in natural layout [k_tokens, D] (fp16) with interleaved ones cols
  - ctx accumulated per q-chunk in natural layout, normalized per branch
    with per-partition (per-q) reciprocals of the denominators
  - output projection from PE-transposed ctx, bias broadcast added on DVE
"""
import math
import os
from contextlib import ExitStack

import numpy as np

import concourse.bass as bass
import concourse.mybir as mybir
import concourse.tile as tile
from concourse import bacc
from concourse.masks import make_identity

B, S, D = 64, 257, 768
H, HD = 12, 64
NT = D // 128          # 6 channel tiles
N_CORES = 8
BPC = B // N_CORES     # batches per core
SCALE = 1.0 / math.sqrt(HD)

# branch geometry: (kernel_size, out_spatial, seq_len)
BRANCHES = [(1, 16, 257), (3, 14, 197), (5, 12, 145)]
SP = S + 1              # fp32r matmuls need an even moving dim

f32 = mybir.dt.float32
f32r = mybir.dt.float32r
f16 = mybir.dt.float16

AF = mybir.ActivationFunctionType
ALU = mybir.AluOpType


def chunks(L, step=128):
    return [(i, min(step, L - i)) for i in range(0, L, step)]


def bcast_free(ap, n):
    """Append a 0-stride free dim of size n to an AP."""
    return bass.AP(tensor=ap.tensor, offset=ap.offset, ap=[*ap.ap, [0, n]])


def build(nbatch=BPC, reps=1):
    nc = bacc.Bacc("TRN2", target_bir_lowering=False, debug=False)

    hid = nc.dram_tensor("hidden", [nbatch, S, D], f32, kind="ExternalInput")
    Wq = nc.dram_tensor("Wq", [D, D], f32, kind="ExternalInput")
    Wk = nc.dram_tensor("Wk", [D, D], f32, kind="ExternalInput")
    Wv = nc.dram_tensor("Wv", [D, D], f32, kind="ExternalInput")
    Wo = nc.dram_tensor("Wo", [D, D], f32, kind="ExternalInput")
    bq = nc.dram_tensor("bq", [D], f32, kind="ExternalInput")
    bk = nc.dram_tensor("bk", [D], f32, kind="ExternalInput")
    bv = nc.dram_tensor("bv", [D], f32, kind="ExternalInput")
    bo = nc.dram_tensor("bo", [D], f32, kind="ExternalInput")
    cw = [nc.dram_tensor(f"cw{i}", [D, k * k], f32, kind="ExternalInput")
          for i, (k, _, _) in enumerate(BRANCHES)]
    out = nc.dram_tensor("out", [nbatch, S, D], f32, kind="ExternalOutput")

    with tile.TileContext(nc) as tc, ExitStack() as ctx:
        persist = ctx.enter_context(tc.tile_pool(name="persist", bufs=1))
        ps_mm = ctx.enter_context(tc.tile_pool(name="ps_mm", bufs=3, space="PSUM"))
        ps_tr = ctx.enter_context(tc.tile_pool(name="ps_tr", bufs=2, space="PSUM"))
        ps_ctx = ctx.enter_context(tc.tile_pool(name="ps_ctx", bufs=1, space="PSUM"))
        batch_pool = ctx.enter_context(tc.tile_pool(name="batch", bufs=1))
        br_pool = ctx.enter_context(tc.tile_pool(name="branch", bufs=2))
        vn_pool = ctx.enter_context(tc.tile_pool(name="vn", bufs=4))
        probs_pool = ctx.enter_context(tc.tile_pool(name="probs", bufs=12))
        small = ctx.enter_context(tc.tile_pool(name="small", bufs=4))
        cacc_pool = ctx.enter_context(tc.tile_pool(name="cacc", bufs=4))

        # ---- constants / weights (once per kernel) ----
        ident = persist.tile([128, 128], f32, tag="ident")
        make_identity(nc, ident[:])

        _wstage = [("hn", 2), ("outsb", 1), ("ctxacc", 2)]

        def load_wT(w_dram, name, scale=1.0, dt_out=f32r):
            wT = persist.tile([128, NT, D], dt_out, tag=name, name=name)
            stages = []
            for half in range(2):
                tag, nb = _wstage[(len(stages) + hash(name)) % len(_wstage)]
                w_nat = batch_pool.tile([128, 3, D], f32, tag=tag, bufs=nb,
                                        name="wnat")
                nc.sync.dma_start(
                    out=w_nat[:],
                    in_=w_dram.ap()[half * 384:(half + 1) * 384, :].rearrange(
                        "(m p) j -> p m j", p=128))
                stages.append(w_nat)
            for half in range(2):
                w_nat = stages[half]
                for m in range(3):
                    mm = half * 3 + m
                    for jt in range(NT):
                        ps = ps_tr.tile([128, 128], f32, tag="tr", name="wtr")
                        nc.tensor.transpose(ps[:], w_nat[:, m, jt * 128:(jt + 1) * 128],
                                            ident[:])
                        nc.scalar.activation(out=wT[:, jt, mm * 128:(mm + 1) * 128],
                                             in_=ps[:], func=AF.Copy, scale=scale)
            return wT

        wqT = load_wT(Wq, "wqT")
        wkT = load_wT(Wk, "wkT", dt_out=f16)
        wvT = load_wT(Wv, "wvT", dt_out=f16)
        woT = load_wT(Wo, "woT", scale=1.0 / 3.0)

        def load_bT(b_dram, name):
            t = persist.tile([128, NT], f32, tag=name)
            nc.sync.dma_start(out=t[:], in_=b_dram.ap().rearrange("(m p) -> p m", p=128))
            return t

        bqT = load_bT(bq, "bqT")
        bkT = load_bT(bk, "bkT")
        # scaled q bias: (x + bq) * SCALE == x*SCALE + bq*SCALE
        bqTs = persist.tile([128, NT], f32, tag="bqTs")
        nc.scalar.mul(out=bqTs[:], in_=bqT[:], mul=SCALE)

        # broadcast biases along partitions for natural-layout adds
        bvb = persist.tile([128, D], f32, tag="bvb")
        nc.gpsimd.dma_start(out=bvb[:], in_=bv.ap().partition_broadcast(128))
        bob = persist.tile([128, D], f32, tag="bob")
        nc.gpsimd.dma_start(out=bob[:], in_=bo.ap().partition_broadcast(128))

        cwt = []
        for i, (k, _, _) in enumerate(BRANCHES):
            t = persist.tile([128, NT, k * k], f32, tag=f"cw{i}")
            nc.sync.dma_start(out=t[:], in_=cw[i].ap().rearrange("(m p) t -> p m t", p=128))
            cwt.append(t)

        # ---- per-batch pipeline ----
        # Emission is software-pipelined at sub-batch granularity: while batch
        # b's attention branches run, the loader/Q-proj/conv stages of batch
        # b+1 are emitted between branch sections so the in-order engines
        # always have independent work queued.

        def stage_load(b):
            hn = batch_pool.tile([128, 3, D], f32, tag="hn", bufs=2, name="hn")
            nc.sync.dma_start(
                out=hn[:, 0:2, :],
                in_=hid.ap()[b, 0:256, :].rearrange("(tt p) c -> p tt c", p=128))
            nc.sync.dma_start(out=hn[0:1, 2, :], in_=hid.ap()[b, 256:257, :])

            hT = batch_pool.tile([128, NT, SP], f32r, tag="hT", bufs=2, name="hT")
            for jt in range(NT):
                for tt, (t0, tsz) in enumerate(chunks(S)):
                    ps = ps_tr.tile([128, 128], f32, tag="tr", name="trp")
                    nc.tensor.transpose(ps[0:128, 0:tsz],
                                        hn[0:tsz, tt, jt * 128:(jt + 1) * 128],
                                        ident[0:tsz, 0:tsz])
                    nc.vector.tensor_copy(hT[:, jt, t0:t0 + tsz], ps[:, 0:tsz])
            nc.scalar.mul(out=hT[:, :, S:SP], in_=hT[:, :, 0:1].bitcast(f32), mul=0.0)
            hTb = batch_pool.tile([128, NT, 256], f16, tag="hTb", bufs=2, name="hTb")
            nc.vector.tensor_copy(hTb[:], hT[:, :, 1:257].bitcast(f32))
            return hT, hTb

        def stage_q(b, hT):
            qT = batch_pool.tile([128, NT, SP], f32r, tag="qT", bufs=2, name="qT")
            for it in range(NT):
                ps = ps_mm.tile([128, 512], f32, tag="mm", name="qps")
                for jt in range(NT):
                    nc.tensor.matmul(ps[:, 0:SP], wqT[:, jt, it * 128:(it + 1) * 128],
                                     hT[:, jt, :], start=(jt == 0), stop=(jt == NT - 1))
                nc.scalar.activation(out=qT[:, it, 0:S], in_=ps[:, 0:S],
                                     func=AF.Identity, bias=bqTs[:, it:it + 1],
                                     scale=SCALE)
            nc.scalar.mul(out=qT[:, :, S:SP], in_=qT[:, :, 0:1].bitcast(f32), mul=0.0)
            return qT

        def stage_xb(b, bi, hT, hTb):
            ksize, osp, L = BRANCHES[bi]
            Lp = L + 1
            xbT = br_pool.tile([128, NT, SP], f16, tag="xbT", bufs=7, name="xbT")
            nc.scalar.mul(out=xbT[:, :, L:Lp], in_=hT[:, :, 0:1].bitcast(f32),
                          mul=0.0)
            nc.scalar.copy(out=xbT[:, :, 0:1], in_=hT[:, :, 0:1].bitcast(f32))
            if ksize == 1:
                for jt in range(NT):
                    nc.scalar.mul(out=xbT[:, jt, 1:257],
                                  in_=hT[:, jt, 1:257].bitcast(f32),
                                  mul=cwt[0][:, jt, 0:1])
                return xbT
            ntap = ksize * ksize
            on_pool = (ksize == 3)
            for jt in range(NT):
                img = hTb[:, jt, :].rearrange("p (r c) -> p r c", r=16)
                acc = None
                for tap in range(ntap):
                    dy, dx = divmod(tap, ksize)
                    win = img[:, dy:dy + osp, dx:dx + osp]
                    nxt = cacc_pool.tile([128, osp, osp], f16, tag="cacc",
                                         name="cacc")
                    w_ap = cwt[bi][:, jt, tap:tap + 1]
                    if acc is None:
                        eng = nc.gpsimd if on_pool else nc.vector
                        eng.tensor_scalar(out=nxt[:], in0=win, scalar1=w_ap,
                                          scalar2=None, op0=ALU.mult)
                    elif on_pool:
                        tmp = cacc_pool.tile([128, osp, osp], f16, tag="ctmp",
                                             name="ctmp")
                        nc.gpsimd.tensor_scalar(out=tmp[:], in0=win, scalar1=w_ap,
                                                scalar2=None, op0=ALU.mult)
                        nc.gpsimd.tensor_tensor(out=nxt[:], in0=tmp[:], in1=acc[:],
                                                op=ALU.add)
                    else:
                        nc.vector.scalar_tensor_tensor(out=nxt[:], in0=win,
                                                       scalar=w_ap, in1=acc[:],
                                                       op0=ALU.mult, op1=ALU.add)
                    acc = nxt
                nc.scalar.copy(out=xbT[:, jt, 1:1 + osp * osp],
                               in_=acc[:].rearrange("p a b -> p (a b)"))
            return xbT

        def branch_attn(b, bi, qT, xbT, ctx_acc):
            ksize, osp, L = BRANCHES[bi]
            kch = chunks(L)
            Lp = L + 1

            kT = br_pool.tile([128, NT, S], f32r, tag="kT", bufs=2, name="kT")
            for it in range(NT):
                ps = ps_mm.tile([128, 512], f32, tag="mm", name="kps")
                for jt in range(NT):
                    nc.tensor.matmul(ps[:, 0:Lp],
                                     wkT[:, jt, it * 128:(it + 1) * 128],
                                     xbT[:, jt, 0:Lp], start=(jt == 0),
                                     stop=(jt == NT - 1))
                nc.scalar.activation(out=kT[:, it, 0:L], in_=ps[:, 0:L],
                                     func=AF.Identity, bias=bkT[:, it:it + 1])

            vn = []
            for (t0, tsz) in kch:
                v = vn_pool.tile([128, H * 65], f16, tag="vn", name="vn")
                v65 = v[:].rearrange("p (h c) -> p h c", c=65)
                nc.gpsimd.memset(v65[0:tsz, :, 64:65], 1.0)
                for half in range(2):
                    ps = ps_mm.tile([128, 512], f32, tag="mm", name="vps")
                    for jt in range(NT):
                        nc.tensor.matmul(
                            ps[0:tsz, 0:384], xbT[:, jt, t0:t0 + tsz],
                            wvT[:, jt, half * 384:(half + 1) * 384],
                            start=(jt == 0), stop=(jt == NT - 1))
                    nc.vector.tensor_tensor(
                        out=v65[0:tsz, half * 6:(half + 1) * 6, 0:64],
                        in0=ps[0:tsz, 0:384].rearrange("p (h c) -> p h c", c=64),
                        in1=bvb[0:tsz, half * 384:(half + 1) * 384].rearrange(
                            "p (h c) -> p h c", c=64),
                        op=ALU.add)
                vn.append(v)

            cps = [None, None]

            def scores_exp(h):
                jt_h, hp = h // 2, (h % 2) * 64
                pts = []
                for kt, (k0, ksz) in enumerate(kch):
                    ps = ps_mm.tile([128, 512], f32, tag="mm", name="sps")
                    nc.tensor.matmul(ps[0:ksz, 0:SP],
                                     kT[hp:hp + 64, jt_h, k0:k0 + ksz],
                                     qT[hp:hp + 64, jt_h, :],
                                     start=True, stop=True)
                    pt = probs_pool.tile([128, S], f16, tag="probs", bufs=12, name="pt")
                    nc.scalar.activation(out=pt[0:ksz, :], in_=ps[0:ksz, 0:S],
                                         func=AF.Exp)
                    pts.append(pt)
                return pts

            pend = {0: scores_exp(0)}
            for h in range(H):
                hg, hh = divmod(h, 6)
                if hh == 0:
                    cps[hg] = ps_ctx.tile([128, 3, 512], f32, tag="ctx",
                                          name="cps")
                if h + 1 < H:
                    pend[h + 1] = scores_exp(h + 1)
                pts = pend.pop(h)
                for qc, (q0, qsz) in enumerate(chunks(S)):
                    for kt, (k0, ksz) in enumerate(kch):
                        nc.tensor.matmul(
                            cps[hg][0:qsz, qc, hh * 65:(hh + 1) * 65],
                            pts[kt][0:ksz, q0:q0 + qsz],
                            vn[kt][0:ksz, h * 65:(h + 1) * 65],
                            start=(kt == 0), stop=(kt == len(kch) - 1),
                            skip_group_check=True)
                if hh == 5:
                    for qc, (q0, qsz) in enumerate(chunks(S)):
                        cp = cps[hg][:, qc, 0:390].rearrange(
                            "p (h c) -> p h c", c=65)
                        r = small.tile([128, 6], f32, tag="recip", name="r")
                        nc.vector.reciprocal(out=r[0:qsz, :],
                                             in_=cp[0:qsz, :, 64:65].rearrange(
                                                 "p h c -> p (h c)"))
                        num = cp[0:qsz, :, 0:64]
                        rb = bcast_free(r[0:qsz, :], 64)
                        d6 = ctx_acc[0:qsz, qc, hg * 384:(hg + 1) * 384].rearrange(
                            "p (h c) -> p h c", c=64)
                        if bi == 0:
                            nc.vector.tensor_tensor(out=d6, in0=num, in1=rb,
                                                    op=ALU.mult)
                        else:
                            tmp = small.tile([128, 6, 64], f32, tag="ntmp",
                                             bufs=2, name="ntmp")
                            nc.vector.tensor_tensor(out=tmp[0:qsz], in0=num,
                                                    in1=rb, op=ALU.mult)
                            nc.gpsimd.tensor_tensor(out=d6, in0=tmp[0:qsz],
                                                    in1=d6, op=ALU.add)

        def epilogue(b, ctx_acc):
            ctxT = batch_pool.tile([128, NT, S], f32r, tag="ctxT", name="ctxT")
            for jt in range(NT):
                for qc, (q0, qsz) in enumerate(chunks(S)):
                    ps = ps_tr.tile([128, 128], f32, tag="tr", name="trc")
                    nc.tensor.transpose(ps[0:128, 0:qsz],
                                        ctx_acc[0:qsz, qc, jt * 128:(jt + 1) * 128],
                                        ident[0:qsz, 0:qsz])
                    nc.scalar.copy(out=ctxT[:, jt, q0:q0 + qsz], in_=ps[:, 0:qsz])

            outsb = batch_pool.tile([128, 3, D], f32, tag="outsb", name="outsb")
            for tt, (t0, tsz) in enumerate(chunks(S)):
                for half in range(2):
                    ps = ps_mm.tile([128, 512], f32, tag="mm", name="ops")
                    for jt in range(NT):
                        nc.tensor.matmul(ps[0:tsz, 0:384], ctxT[:, jt, t0:t0 + tsz],
                                         woT[:, jt, half * 384:(half + 1) * 384],
                                         start=(jt == 0), stop=(jt == NT - 1))
                    nc.vector.tensor_tensor(
                        out=outsb[0:tsz, tt, half * 384:(half + 1) * 384],
                        in0=ps[0:tsz, 0:384],
                        in1=bob[0:tsz, half * 384:(half + 1) * 384], op=ALU.add)

            nc.sync.dma_start(
                out=out.ap()[b, 0:256, :].rearrange("(tt p) c -> p tt c", p=128),
                in_=outsb[:, 0:2, :])
            nc.sync.dma_start(out=out.ap()[b, 256:257, :], in_=outsb[0:1, 2, :])

        loop_cm = tc.For_i(0, reps, 1) if reps > 1 else None
        if loop_cm is not None:
            loop_cm.__enter__()

        # state per batch: (hT, hTb, qT, xbTs)
        st = {}

        def full_prologue(b):
            hT, hTb = stage_load(b)
            qT = stage_q(b, hT)
            xbTs = [stage_xb(b, bi, hT, hTb) for bi in range(3)]
            st[b] = (hT, hTb, qT, xbTs)

        full_prologue(0)
        prev = None
        for b in range(nbatch):
            hT, hTb, qT, xbTs = st[b]
            ctx_acc = batch_pool.tile([128, 3, D], f32, tag="ctxacc", bufs=2,
                                      name="ctxacc")
            for bi in range(3):
                # interleave next batch's prologue stages between branches
                if b + 1 < nbatch:
                    if bi == 0:
                        nhT, nhTb = stage_load(b + 1)
                    elif bi == 1:
                        nqT = stage_q(b + 1, nhT)
                        nxb = [stage_xb(b + 1, 0, nhT, nhTb),
                               stage_xb(b + 1, 1, nhT, nhTb)]
                    else:
                        nxb.append(stage_xb(b + 1, 2, nhT, nhTb))
                        st[b + 1] = (nhT, nhTb, nqT, nxb)
                if bi == 1 and prev is not None:
                    epilogue(prev[0], prev[1])
                branch_attn(b, bi, qT, xbTs[bi], ctx_acc)
            st.pop(b, None)
            prev = (b, ctx_acc)
        epilogue(prev[0], prev[1])
        if loop_cm is not None:
            loop_cm.__exit__(None, None, None)

    nc.compile()
    return nc


_COMPILED = {}


def _get_program(nbatch):
    if nbatch not in _COMPILED:
        _COMPILED[nbatch] = build(nbatch)
    return _COMPILED[nbatch]


def make_in_maps(inputs, nbatch=BPC, n_cores=N_CORES):
    h = np.ascontiguousarray(np.asarray(inputs["hidden_states"], dtype=np.float32))
    common = {
        "Wq": np.asarray(inputs["Wq"], np.float32),
        "Wk": np.asarray(inputs["Wk"], np.float32),
        "Wv": np.asarray(inputs["Wv"], np.float32),
        "Wo": np.asarray(inputs["Wo"], np.float32),
        "bq": np.asarray(inputs["bq"], np.float32),
        "bk": np.asarray(inputs["bk"], np.float32),
        "bv": np.asarray(inputs["bv"], np.float32),
        "bo": np.asarray(inputs["bo"], np.float32),
        "cw0": np.ascontiguousarray(np.asarray(inputs["conv1_w"], np.float32).reshape(D, 1)),
        "cw1": np.ascontiguousarray(np.asarray(inputs["conv2_w"], np.float32).reshape(D, 9)),
        "cw2": np.ascontiguousarray(np.asarray(inputs["conv3_w"], np.float32).reshape(D, 25)),
    }
    in_maps = []
    for c in range(n_cores):
        m = dict(common)
        m["hidden"] = np.ascontiguousarray(h[c * nbatch:(c + 1) * nbatch])
        in_maps.append(m)
    return in_maps


def kernel(**inputs) -> np.ndarray:
    from concourse.bass_utils import run_bass_kernel_spmd
    nc = _get_program(BPC)
    in_maps = make_in_maps(inputs)
    res = run_bass_kernel_spmd(nc, in_maps, list(range(N_CORES)))
    return np.concatenate([res.results[c]["out"] for c in range(N_CORES)], axis=0)
